# revision 1
# baseline (speedup 1.0000x reference)
"""Bass/Trainium2 kernel for nn_CustomPooling (segment_reduce, masked mean pooling).

Reference computation:
  hs = mean(hidden_states[-4:], axis=0)                      # [B,S,H]
  valid = before_pad & ~CLS & ~SEP & attention
  term_mean = sum_s(hs * term_mask) / sum(term_mask)         # [B,H]
  text_mean = sum_s(hs * text_mask) / sum(text_mask)         # [B,H]
  out = concat([term_mean, text_mean], -1)                   # [B,2H]

Strategy:
  - Only the last 4 layers are ever read (201MB of the 654MB input).
  - The [B,S] int masks reduce to binary {0,1} per-(b,s) weights; the
    1/(4*count) scale is applied to the tiny [B,2H] result on the host, so
    the device work is a pure masked sum over (layer, s):
      acc[b, m*H + h] = sum_{l,s} hs[l,b,s,h] * mask[b,s,m]
  - That reduction is a TensorE matmul with the [128,2] binary mask slice
    stationary and hs [128, N] moving, accumulated in fp32 PSUM over
    4 s-chunks x 4 layers. Data is shipped as fp16 ({0,1} masks are exact;
    hs quantization gives ~4e-4 rel err) which halves DMA bytes and runs
    the PE at full (1 col/cycle) rate instead of the 4x-slower fp32 path.
  - Data parallel over B: 8 cores x 4 batches, no collectives.
  - Host pre-swizzles each (batch, layer-pair) into one contiguous
    [128, 6152] fp16 blob (its own weight copy appended) so each tile is
    ONE ~1.57MB DMA and every matmul waits on exactly one DMA semaphore
    (this toolchain accepts a single sync wait per instruction). The 8 hs
    DMAs alternate between the two HWDGE rings (sync/scalar) to keep all
    16 SDMA engines latency-hidden; the tiny output store uses SWDGE to
    avoid wrapping the 8 HWDGE semaphore lanes.
"""

import os

import numpy as np

# Hardcoded problem shape (kernel.py must be self-contained).
L, B, S, H = 13, 32, 512, 768
N_LAYERS = 4          # layers -4..-1
N_CORES = 8
B_SHARD = B // N_CORES          # 4 batches per core
N_CHUNKS = S // 128             # 4 s-chunks of 128 (PE contraction dim)
W_COLS = N_CHUNKS * 2                    # 8
# Bulk batches (0..2) ship as two half-blobs (2 layers each); the tail
# batch ships as four quarter-blobs (1 layer) so the last-arriving tile
# only needs ~1.4us of matmuls after the final DMA lands.
HALF_HS = 2 * N_CHUNKS * H               # 6144
HALF_COLS = HALF_HS + W_COLS             # 6152
QUART_HS = N_CHUNKS * H                  # 3072
QUART_COLS = QUART_HS + W_COLS           # 3080
CLS_ID, SEP_ID, PAD_ID = 101, 102, 0

_CACHED = {}


def _build_bass():
    import concourse.bass as bass
    import concourse.tile as tile
    from concourse import mybir

    f16 = mybir.dt.float16
    f32 = mybir.dt.float32
    nc = bass.Bass()

    # Per-core inputs (host-preswizzled fp16 blobs, masks appended to each):
    #   hsa[b, hf, p, l2*3072 + c*768 + h], b in 0..2  (two half-blobs each)
    #   hsb[l, p, c*768 + h]                           (batch 3, per layer)
    hsa = nc.dram_tensor("hsa", [3, 2, 128, HALF_COLS], f16, kind="ExternalInput")
    hsb = nc.dram_tensor("hsb", [N_LAYERS, 128, QUART_COLS], f16, kind="ExternalInput")
    out = nc.dram_tensor("out", [B_SHARD, 2 * H], f32, kind="ExternalOutput")

    dma_idx = [0]

    def hs_dma(out_ap, in_ap):
        eng = nc.sync if dma_idx[0] % 2 == 0 else nc.scalar
        dma_idx[0] += 1
        eng.dma_start(out=out_ap, in_=in_ap)

    with tile.TileContext(nc) as tc:
        with (
            tc.tile_pool(name="hs_pool", bufs=6) as hs_pool,
            tc.tile_pool(name="hsq_pool", bufs=4) as hsq_pool,
            tc.tile_pool(name="out_pool", bufs=1) as out_pool,
            tc.tile_pool(name="psum", bufs=4, space="PSUM") as psum_pool,
        ):
            out_tile = out_pool.tile([2, B_SHARD * H], f32)

            for b in range(B_SHARD):
                # (lhsT, rhs_A, rhs_B) per (layer, chunk); weights live in
                # whichever tile the rhs comes from so each matmul waits on
                # exactly one DMA.
                mm_args = []
                if b < 3:
                    for hf in range(2):
                        t = hs_pool.tile([128, HALF_COLS], f16, tag="hs")
                        hs_dma(t[:], hsa[b, hf])
                        for l2 in range(2):
                            for c in range(N_CHUNKS):
                                lhsT = t[:, HALF_HS + c * 2 : HALF_HS + c * 2 + 2]
                                col0 = (l2 * N_CHUNKS + c) * H
                                mm_args.append((lhsT, t[:, col0 : col0 + 512],
                                                t[:, col0 + 512 : col0 + H]))
                else:
                    for l in range(N_LAYERS):
                        t = hsq_pool.tile([128, QUART_COLS], f16, tag="hsq")
                        hs_dma(t[:], hsb[l])
                        for c in range(N_CHUNKS):
                            lhsT = t[:, QUART_HS + c * 2 : QUART_HS + c * 2 + 2]
                            col0 = c * H
                            mm_args.append((lhsT, t[:, col0 : col0 + 512],
                                            t[:, col0 + 512 : col0 + H]))

                # Interleaved bank-A (N=512) / bank-B (N=256) groups in
                # separate PSUM banks; the A copy only waits on the A group
                # so it overlaps the final B matmul.
                psum_a = psum_pool.tile([2, 512], f32, tag="psum_a")
                psum_b = psum_pool.tile([2, H - 512], f32, tag="psum_b")
                n = len(mm_args)
                for i, (lhsT, rhs_a, rhs_b) in enumerate(mm_args):
                    nc.tensor.matmul(psum_a[:, :], lhsT, rhs_a,
                                     start=i == 0, stop=i == n - 1)
                    nc.tensor.matmul(psum_b[:, :], lhsT, rhs_b,
                                     start=i == 0, stop=i == n - 1)
                nc.vector.tensor_copy(
                    out=out_tile[:, b * H : b * H + 512], in_=psum_a[:, :]
                )
                nc.vector.tensor_copy(
                    out=out_tile[:, b * H + 512 : (b + 1) * H], in_=psum_b[:, :]
                )
                if b == 2:
                    # Bulk store (b0..b2) hides under b3's matmuls. Same
                    # SWDGE ring as the final store -> ring FIFO orders it
                    # before the final store's completion sem.
                    nc.gpsimd.dma_start(
                        out=out[0:3].rearrange("b (m h) -> m b h", m=2),
                        in_=out_tile[:, 0 : 3 * H].rearrange(
                            "m (b h) -> m b h", b=3
                        ),
                    )

            # Final (b3) store. SWDGE (gpsimd): the 10 hs DMAs wrap the 8
            # HWDGE sem lanes; more HWDGE DMAs would need a 2nd sync wait.
            nc.gpsimd.dma_start(
                out=out[3:4].rearrange("b (m h) -> m b h", m=2),
                in_=out_tile[:, 3 * H : 4 * H].rearrange(
                    "m (b h) -> m b h", b=1
                ),
            )

    _fix_drain_waits(nc)
    return nc


def _fix_drain_waits(nc):
    """This container's walrus accepts only ONE sync wait per instruction;
    Tile's exit drain aggregates one wait per live semaphore. In this kernel
    every semaphore except the final out-DMA's is transitively ordered before
    the drain (matmuls wait on hs DMAs -> PE; copies wait on PE -> DVE; the
    out DMA waits on DVE; the drain runs after on the same SP queue), so the
    drain only truly needs the out-DMA completion wait.
    """
    import bass_rust

    f = nc.m.functions[0]
    # update-sem of the last DMACopy in program order (the out store)
    last_dma_sem = None
    for bb in f.blocks:
        for ins in bb.instructions:
            if type(ins).__name__ == "InstDMACopy":
                ups = ins.sync_info.on_update
                if ups:
                    last_dma_sem = ups[-1].ant_name

    for bb in f.blocks:
        for ins in bb.instructions:
            if type(ins).__name__ != "InstDrain":
                continue
            si = ins.sync_info
            if si is None:
                continue
            waits = list(si.on_wait)
            if len(waits) <= 1:
                continue
            keep = [w for w in waits if w.ant_name == last_dma_sem]
            assert len(keep) == 1, (last_dma_sem, [w.ant_name for w in waits])
            ins.sync_info = bass_rust.SyncInfo(
                on_wait=keep, on_update=list(si.on_update)
            )


def _host_masks(input_ids, attention_mask, token_type_ids):
    ids = np.asarray(input_ids)
    am = np.asarray(attention_mask)
    tt = np.asarray(token_type_ids)

    not_pad = ids != PAD_ID
    before_pad = np.cumprod(not_pad.astype(np.int64), axis=1).astype(bool)
    valid = before_pad & (ids != CLS_ID) & (ids != SEP_ID) & (am == 1)
    term = valid & (tt == 0)
    text = valid & (tt == 1)
    masks = np.stack([term, text], axis=-1)  # [B, S, 2] bool
    counts = masks.sum(axis=1).astype(np.float64)  # [B, 2]
    return masks.astype(np.float16), counts


def _compensated_fp16(hs4, masks):
    """Quantize to fp16 with error diffusion along the reduction axis: the
    rounding residual of each masked element is carried into the next masked
    element of the same (b, h) chain, so each group's quantization errors
    telescope to ~1 ulp instead of a sqrt(N) random walk. Device-side sum
    order doesn't matter -- only the group SUM of the quantized values.
    """
    q = hs4.astype(np.float16)  # [4, B, S, H]
    gate = masks.any(axis=-1)  # [B, S] -- element participates in some group
    carry = np.zeros((B, H), dtype=np.float32)
    for l in range(N_LAYERS):
        for s in range(S):
            g = gate[:, s]
            if not g.any():
                continue
            t = hs4[l, :, s, :] + carry
            qv = t.astype(np.float16)
            q[l, :, s, :] = np.where(g[:, None], qv, q[l, :, s, :])
            carry = np.where(g[:, None], t - qv.astype(np.float32), carry)
    return q


def kernel(hidden_states, input_ids, attention_mask, token_type_ids):
    from concourse.bass_utils import run_bass_kernel_spmd

    hs_full = np.asarray(hidden_states)
    masks, counts = _host_masks(input_ids, attention_mask, token_type_ids)

    hs4 = _compensated_fp16(
        hs_full[L - N_LAYERS :].astype(np.float32), masks.astype(bool)
    )  # [4, B, S, H] fp16

    # Half-blobs [B, hf, p, (l2 c h)] and quarter-blobs [B, l, p, (c h)]
    half = np.empty((B, 2, 128, HALF_COLS), dtype=np.float16)
    half[:, :, :, :HALF_HS] = (
        hs4.reshape(2, 2, B, N_CHUNKS, 128, H)
        .transpose(2, 0, 4, 1, 3, 5)
        .reshape(B, 2, 128, HALF_HS)
    )
    quart = np.empty((B, N_LAYERS, 128, QUART_COLS), dtype=np.float16)
    quart[:, :, :, :QUART_HS] = (
        hs4.reshape(N_LAYERS, B, N_CHUNKS, 128, H)
        .transpose(1, 0, 3, 2, 4)
        .reshape(B, N_LAYERS, 128, QUART_HS)
    )
    wv = masks.reshape(B, N_CHUNKS, 128, 2).transpose(0, 2, 1, 3).reshape(
        B, 128, W_COLS
    )
    half[:, :, :, HALF_HS:] = wv[:, None, :, :]
    quart[:, :, :, QUART_HS:] = wv[:, None, :, :]

    in_maps = [
        {
            "hsa": half[i * B_SHARD : i * B_SHARD + 3],
            "hsb": quart[i * B_SHARD + 3],
        }
        for i in range(N_CORES)
    ]

    if "nc" not in _CACHED:
        _CACHED["nc"] = _build_bass()
    nc = _CACHED["nc"]

    trace = os.environ.get("KERNEL_TRACE", "0") == "1"
    if trace:
        _install_ntff_hook_shim()
    tmpdir = os.environ.get("KERNEL_TMPDIR") or None
    res = run_bass_kernel_spmd(
        nc, in_maps, core_ids=list(range(N_CORES)), trace=trace, tmpdir=tmpdir
    )
    kernel.last_results = res

    acc = np.concatenate([r["out"] for r in res.results], axis=0)  # [B, 2H]
    # Apply the masked-mean normalization (exact f64 scale, mirrors the
    # reference's sum/count including inf/nan semantics for count==0).
    with np.errstate(divide="ignore", invalid="ignore"):
        scale = 1.0 / (N_LAYERS * counts)  # [B, 2]
    out = acc.reshape(B, 2, H) * scale[:, :, None]
    return out.reshape(B, 2 * H).astype(np.float32)


def _install_ntff_hook_shim():
    """The container's antenv stub lacks axon_hooks, which silently disables
    NTFF profiling under trace=True. Recreate it: a tiny get/set registry plus
    the ctypes hook into libaxon_pjrt.so (same as trn_boot's installer)."""
    import contextlib
    import ctypes
    import sys
    import types

    if "antenv.axon_hooks" in sys.modules:
        return
    so_path = "/opt/axon/libaxon_pjrt.so"
    try:
        lib = ctypes.CDLL(so_path)
    except OSError:
        return
    if not hasattr(lib, "axon_start_nrt_profile"):
        return
    lib.axon_start_nrt_profile.argtypes = [
        ctypes.POINTER(ctypes.c_int64),
        ctypes.c_size_t,
    ]
    lib.axon_start_nrt_profile.restype = ctypes.c_int64
    lib.axon_stop_nrt_profile.argtypes = [ctypes.c_char_p]
    lib.axon_stop_nrt_profile.restype = ctypes.c_int64

    @contextlib.contextmanager
    def _hook(output_dir, device_ids):
        import jax

        jax.devices()
        if device_ids:
            ids = (ctypes.c_int64 * len(device_ids))(*device_ids)
            rc = lib.axon_start_nrt_profile(ids, len(device_ids))
        else:
            rc = lib.axon_start_nrt_profile(None, 0)
        if rc != 0:
            raise RuntimeError(f"axon_start_nrt_profile rc={rc}")
        try:
            yield
        finally:
            n = lib.axon_stop_nrt_profile(str(output_dir).encode())
            print(f"profile: {n} file(s) written to {output_dir}", file=sys.stderr)

    mod = types.ModuleType("antenv.axon_hooks")
    _state = {"hook": _hook}
    mod.set_axon_ntff_profile_hook = lambda h: _state.__setitem__("hook", h)
    mod.get_axon_ntff_profile_hook = lambda: _state["hook"]
    sys.modules["antenv.axon_hooks"] = mod
    import antenv

    antenv.axon_hooks = mod



# revision 5
# speedup vs baseline: 1.5103x; 1.5103x over previous
"""Bass/Trainium2 kernel for nn_CustomPooling (segment_reduce, masked mean pooling).

Reference computation:
  hs = mean(hidden_states[-4:], axis=0)                      # [B,S,H]
  valid = before_pad & ~CLS & ~SEP & attention
  term_mean = sum_s(hs * term_mask) / sum(term_mask)         # [B,H]
  text_mean = sum_s(hs * text_mask) / sum(text_mask)         # [B,H]
  out = concat([term_mean, text_mean], -1)                   # [B,2H]

Strategy (v2, fp8):
  - Only the last 4 layers are ever read (201MB of the 654MB input).
  - The [B,S] int masks reduce to binary {0,1} per-(b,s) weights; the
    1/(4*count) scale is applied to the tiny [B,2H] result on the host, so
    the device work is a pure masked sum over (layer, s):
      acc[b, m*H + h] = sum_{l,s} hs[l,b,s,h] * mask[b,s,m]
  - Data ships as fp8 e4m3 ({0,1} masks exact; hs quantized with per-group
    error diffusion so each group's rounding errors telescope to ~1 ulp).
    This halves DMA bytes vs the fp16 version: 6.3MB/core, ~15us at the
    ~427GB/s the 16 SDMA engines sustain.
  - The reduction is a TensorE matmul in fp8 DoubleRow perf mode: one
    instruction contracts TWO 128-row s-chunks (K=256) at the double-pumped
    fp8 rate. lhsT = [128,2,2] mask slice, rhs = [128,2,N] hs slice.
  - Data parallel over B: 8 cores x 4 batches, no collectives.
  - Host pre-swizzles each (batch, layer-pair) into one contiguous
    [128, 8, 770] fp8 half-blob (per (layer,chunk) unit: 768 hs cols + its
    own 2 mask cols) so every matmul waits on exactly one DMA semaphore and
    per-partition DMA rows are 6160B (full 26GB/s per-engine packet rate;
    3KB rows drop to ~18GB/s). 8 half-blob DMAs alternate between the two
    HWDGE rings (sync/scalar) to keep all 16 SDMA engines saturated.
  - PSUM->SBUF copies cast to fp16 (2x DVE rate, ~1e-4 rel err); the single
    output store is one contiguous HWDGE DMA of the m-major [2,B/8*H] tile;
    the host transposes to [B/8, 2H] and applies the f64 1/(4*count) scale.
"""

import os

import numpy as np

# Hardcoded problem shape (kernel.py must be self-contained).
L, B, S, H = 13, 32, 512, 768
N_LAYERS = 4          # layers -4..-1
N_CORES = 8
B_SHARD = B // N_CORES          # 4 batches per core
N_CHUNKS = S // 128             # 4 s-chunks of 128 (PE contraction dim)
CH_COLS = H + 16                # 784 (16-aligned: DoubleRow needs k-tile step%16==0);
                                # per unit: 768 hs cols, 2 mask cols, 14 pad
HALF_UNITS = 2 * N_CHUNKS       # 8 units per half-blob (2 layers x 4 chunks)
CLS_ID, SEP_ID, PAD_ID = 101, 102, 0

_CACHED = {}


def _build_bass():
    import concourse.bass as bass
    import concourse.tile as tile
    from concourse import mybir

    f8 = mybir.dt.float8e4
    f16 = mybir.dt.float16
    f32 = mybir.dt.float32
    DR = mybir.MatmulPerfMode.DoubleRow
    nc = bass.Bass()

    # Per-core input: host-preswizzled fp8 half-blobs, one per (batch, 2
    # layers): hsh[b, hf, p, l2*4 + c, 0:768] = hs, [..., 768:770] = masks.
    hsh = nc.dram_tensor(
        "hsh", [B_SHARD, 2, 128, HALF_UNITS, CH_COLS], f8, kind="ExternalInput"
    )
    # m-major output; host transposes to [B_SHARD, 2H].
    out = nc.dram_tensor("out", [2, B_SHARD * H], f16, kind="ExternalOutput")

    dma_idx = [0]

    def hs_dma(out_ap, in_ap):
        eng = nc.sync if dma_idx[0] % 2 == 0 else nc.scalar
        dma_idx[0] += 1
        eng.dma_start(out=out_ap, in_=in_ap)

    with tile.TileContext(nc) as tc:
        with (
            tc.tile_pool(name="hs_pool", bufs=8) as hs_pool,
            tc.tile_pool(name="out_pool", bufs=1) as out_pool,
            tc.tile_pool(name="psum", bufs=4, space="PSUM") as psum_pool,
        ):
            out_tile = out_pool.tile([2, B_SHARD * H], f16)

            for b in range(B_SHARD):
                # Two half-blobs (2 layers each) per batch, one per ring.
                mm_args = []
                for hf in range(2):
                    t = hs_pool.tile([128, HALF_UNITS, CH_COLS], f8, tag="hs")
                    hs_dma(t[:], hsh[b, hf])
                    for l2 in range(2):
                        for p in range(2):  # chunk pairs (DoubleRow: K=256)
                            u = l2 * N_CHUNKS + 2 * p
                            lhsT = t[:, u : u + 2, H : H + 2]  # step 784 %16==0
                            mm_args.append(
                                (lhsT, t[:, u : u + 2, 0:512],
                                 t[:, u : u + 2, 512:H])
                            )

                # Interleaved bank-A (N=512) / bank-B (N=256) groups in
                # separate PSUM banks so the two accumulations pipeline.
                psum_a = psum_pool.tile([2, 512], f32, tag="psum_a")
                psum_b = psum_pool.tile([2, H - 512], f32, tag="psum_b")
                n = len(mm_args)
                for i, (lhsT, rhs_a, rhs_b) in enumerate(mm_args):
                    nc.tensor.matmul(psum_a[:, :], lhsT, rhs_a,
                                     start=i == 0, stop=i == n - 1,
                                     perf_mode=DR)
                    nc.tensor.matmul(psum_b[:, :], lhsT, rhs_b,
                                     start=i == 0, stop=i == n - 1,
                                     perf_mode=DR)
                nc.vector.tensor_copy(
                    out=out_tile[:, b * H : b * H + 512], in_=psum_a[:, :]
                )
                nc.vector.tensor_copy(
                    out=out_tile[:, b * H + 512 : (b + 1) * H], in_=psum_b[:, :]
                )

            # Single contiguous output store (12KB, two 6KB packets) on
            # SWDGE: the 8 hs DMAs fill all 8 HWDGE sem lanes; a 9th HWDGE
            # DMA would wrap a lane and need a 2nd sync wait (walrus allows
            # only one).
            nc.gpsimd.dma_start(out=out[:], in_=out_tile[:])

    _fix_drain_waits(nc)
    return nc


def _fix_drain_waits(nc):
    """This container's walrus accepts only ONE sync wait per instruction;
    Tile's exit drain aggregates one wait per live semaphore. In this kernel
    every semaphore except the final out-DMA's is transitively ordered before
    the drain (matmuls wait on hs DMAs -> PE; copies wait on PE -> DVE; the
    out DMA waits on DVE; the drain runs after on the same SP queue), so the
    drain only truly needs the out-DMA completion wait.
    """
    import bass_rust

    f = nc.m.functions[0]
    # update-sem of the last DMACopy in program order (the out store)
    last_dma_sem = None
    for bb in f.blocks:
        for ins in bb.instructions:
            if type(ins).__name__ == "InstDMACopy":
                ups = ins.sync_info.on_update
                if ups:
                    last_dma_sem = ups[-1].ant_name

    for bb in f.blocks:
        for ins in bb.instructions:
            if type(ins).__name__ != "InstDrain":
                continue
            si = ins.sync_info
            if si is None:
                continue
            waits = list(si.on_wait)
            if len(waits) <= 1:
                continue
            keep = [w for w in waits if w.ant_name == last_dma_sem]
            assert len(keep) == 1, (last_dma_sem, [w.ant_name for w in waits])
            ins.sync_info = bass_rust.SyncInfo(
                on_wait=keep, on_update=list(si.on_update)
            )


def _host_masks(input_ids, attention_mask, token_type_ids):
    ids = np.asarray(input_ids)
    am = np.asarray(attention_mask)
    tt = np.asarray(token_type_ids)

    not_pad = ids != PAD_ID
    before_pad = np.cumprod(not_pad.astype(np.int64), axis=1).astype(bool)
    valid = before_pad & (ids != CLS_ID) & (ids != SEP_ID) & (am == 1)
    term = valid & (tt == 0)
    text = valid & (tt == 1)
    masks = np.stack([term, text], axis=-1)  # [B, S, 2] bool
    counts = masks.sum(axis=1).astype(np.float64)  # [B, 2]
    return masks, counts


def _compensated_fp8(hs4, masks, f8):
    """Quantize to fp8 e4m3 with per-group error diffusion along the
    reduction axis: the rounding residual of each masked element is carried
    into the next masked element of the SAME group's (b, h) chain, so each
    group's quantization errors telescope to ~1 ulp instead of a sqrt(N)
    random walk. Device-side sum order doesn't matter -- only the group SUM
    of the quantized values.
    """
    q = hs4.astype(f8)  # [4, B, S, H]; unmasked positions: plain rounding
    g0_all = masks[:, :, 0]
    g1_all = masks[:, :, 1]
    carry = np.zeros((2, B, H), dtype=np.float32)
    for l in range(N_LAYERS):
        for s in range(S):
            g0 = g0_all[:, s]
            g1 = g1_all[:, s]
            if not (g0.any() or g1.any()):
                continue
            gany = (g0 | g1)[:, None]
            c = np.where(g0[:, None], carry[0], carry[1])
            t = hs4[l, :, s, :] + c
            qv = t.astype(f8)
            q[l, :, s, :] = np.where(gany, qv, q[l, :, s, :])
            resid = t - qv.astype(np.float32)
            carry[0] = np.where(g0[:, None], resid, carry[0])
            carry[1] = np.where(g1[:, None], resid, carry[1])
    return q


def kernel(hidden_states, input_ids, attention_mask, token_type_ids):
    import ml_dtypes
    from concourse.bass_utils import run_bass_kernel_spmd

    f8 = ml_dtypes.float8_e4m3fn  # same encoding as TRN e4m3 for |x| <= 240

    hs_full = np.asarray(hidden_states)
    masks, counts = _host_masks(input_ids, attention_mask, token_type_ids)

    q = _compensated_fp8(
        hs_full[L - N_LAYERS :].astype(np.float32), masks, f8
    )  # [4, B, S, H] fp8

    # Half-blobs [B, hf, p, (l2 c), col]: hs cols 0:768, mask cols 768:770.
    blob = np.empty((B, 2, 128, HALF_UNITS, CH_COLS), dtype=f8)
    blob[..., :H] = (
        q.reshape(2, 2, B, N_CHUNKS, 128, H)
        .transpose(2, 0, 4, 1, 3, 5)
        .reshape(B, 2, 128, HALF_UNITS, H)
    )
    m8 = masks.astype(f8).reshape(B, N_CHUNKS, 128, 2).transpose(0, 2, 1, 3)
    blob[..., H : H + 2] = np.broadcast_to(
        m8[:, None, :, None, :, :], (B, 2, 128, 2, N_CHUNKS, 2)
    ).reshape(B, 2, 128, HALF_UNITS, 2)
    blob[..., H + 2 :] = np.zeros((), dtype=f8)  # pad cols, never read

    in_maps = [
        {"hsh": blob[i * B_SHARD : (i + 1) * B_SHARD]} for i in range(N_CORES)
    ]

    if "nc" not in _CACHED:
        _CACHED["nc"] = _build_bass()
    nc = _CACHED["nc"]

    trace = os.environ.get("KERNEL_TRACE", "0") == "1"
    if trace:
        _install_ntff_hook_shim()
    tmpdir = os.environ.get("KERNEL_TMPDIR") or None
    res = run_bass_kernel_spmd(
        nc, in_maps, core_ids=list(range(N_CORES)), trace=trace, tmpdir=tmpdir
    )
    kernel.last_results = res

    # [2, B_SHARD*H] fp16 per core -> [B, 2, H] f32
    acc = np.concatenate(
        [r["out"].reshape(2, B_SHARD, H).transpose(1, 0, 2) for r in res.results],
        axis=0,
    ).astype(np.float32)
    # Apply the masked-mean normalization (exact f64 scale, mirrors the
    # reference's sum/count including inf/nan semantics for count==0).
    with np.errstate(divide="ignore", invalid="ignore"):
        scale = 1.0 / (N_LAYERS * counts)  # [B, 2]
    out = acc * scale[:, :, None]
    return out.reshape(B, 2 * H).astype(np.float32)


def _install_ntff_hook_shim():
    """The container's antenv stub lacks axon_hooks, which silently disables
    NTFF profiling under trace=True. Recreate it: a tiny get/set registry plus
    the ctypes hook into libaxon_pjrt.so (same as trn_boot's installer)."""
    import contextlib
    import ctypes
    import sys
    import types

    if "antenv.axon_hooks" in sys.modules:
        return
    so_path = "/opt/axon/libaxon_pjrt.so"
    try:
        lib = ctypes.CDLL(so_path)
    except OSError:
        return
    if not hasattr(lib, "axon_start_nrt_profile"):
        return
    lib.axon_start_nrt_profile.argtypes = [
        ctypes.POINTER(ctypes.c_int64),
        ctypes.c_size_t,
    ]
    lib.axon_start_nrt_profile.restype = ctypes.c_int64
    lib.axon_stop_nrt_profile.argtypes = [ctypes.c_char_p]
    lib.axon_stop_nrt_profile.restype = ctypes.c_int64

    @contextlib.contextmanager
    def _hook(output_dir, device_ids):
        import jax

        jax.devices()
        if device_ids:
            ids = (ctypes.c_int64 * len(device_ids))(*device_ids)
            rc = lib.axon_start_nrt_profile(ids, len(device_ids))
        else:
            rc = lib.axon_start_nrt_profile(None, 0)
        if rc != 0:
            raise RuntimeError(f"axon_start_nrt_profile rc={rc}")
        try:
            yield
        finally:
            n = lib.axon_stop_nrt_profile(str(output_dir).encode())
            print(f"profile: {n} file(s) written to {output_dir}", file=sys.stderr)

    mod = types.ModuleType("antenv.axon_hooks")
    _state = {"hook": _hook}
    mod.set_axon_ntff_profile_hook = lambda h: _state.__setitem__("hook", h)
    mod.get_axon_ntff_profile_hook = lambda: _state["hook"]
    sys.modules["antenv.axon_hooks"] = mod
    import antenv

    antenv.axon_hooks = mod


# revision 9
# speedup vs baseline: 1.6503x; 1.0927x over previous
"""Bass/Trainium2 kernel for nn_CustomPooling (segment_reduce, masked mean pooling).

Reference computation:
  hs = mean(hidden_states[-4:], axis=0)                      # [B,S,H]
  valid = before_pad & ~CLS & ~SEP & attention
  term_mean = sum_s(hs * term_mask) / sum(term_mask)         # [B,H]
  text_mean = sum_s(hs * text_mask) / sum(text_mask)         # [B,H]
  out = concat([term_mean, text_mean], -1)                   # [B,2H]

Strategy (v2, fp8):
  - Only the last 4 layers are ever read (201MB of the 654MB input).
  - The [B,S] int masks reduce to binary {0,1} per-(b,s) weights; the
    1/(4*count) scale is applied to the tiny [B,2H] result on the host, so
    the device work is a pure masked sum over (layer, s):
      acc[b, m*H + h] = sum_{l,s} hs[l,b,s,h] * mask[b,s,m]
  - Data ships as fp8 e4m3 ({0,1} masks exact; hs quantized with per-group
    error diffusion so each group's rounding errors telescope to ~1 ulp).
    This halves DMA bytes vs the fp16 version: 6.3MB/core, ~15us at the
    ~427GB/s the 16 SDMA engines sustain.
  - The reduction is a TensorE matmul in fp8 DoubleRow perf mode: one
    instruction contracts TWO 128-row s-chunks (K=256) at the double-pumped
    fp8 rate. lhsT = [128,2,2] mask slice, rhs = [128,2,N] hs slice.
  - Data parallel over B: 8 cores x 4 batches, no collectives.
  - Host pre-swizzles each (batch, layer-pair) into one contiguous
    [128, 8, 770] fp8 half-blob (per (layer,chunk) unit: 768 hs cols + its
    own 2 mask cols) so every matmul waits on exactly one DMA semaphore and
    per-partition DMA rows are 6160B (full 26GB/s per-engine packet rate;
    3KB rows drop to ~18GB/s). 8 half-blob DMAs alternate between the two
    HWDGE rings (sync/scalar) to keep all 16 SDMA engines saturated.
  - PSUM->SBUF copies cast to fp16 (2x DVE rate, ~1e-4 rel err); the single
    output store is one contiguous HWDGE DMA of the m-major [2,B/8*H] tile;
    the host transposes to [B/8, 2H] and applies the f64 1/(4*count) scale.
"""

import os

import numpy as np

# Hardcoded problem shape (kernel.py must be self-contained).
L, B, S, H = 13, 32, 512, 768
N_LAYERS = 4          # layers -4..-1
N_CORES = 8
B_SHARD = B // N_CORES          # 4 batches per core
N_CHUNKS = S // 128             # 4 s-chunks of 128 (PE contraction dim)
CH_COLS = H + 16                # 784 (16-aligned: DoubleRow needs k-tile step%16==0);
                                # per unit: 768 hs cols, 2 mask cols, 14 pad
HALF_UNITS = 2 * N_CHUNKS       # 8 units per half-blob (2 layers x 4 chunks)
CLS_ID, SEP_ID, PAD_ID = 101, 102, 0

_CACHED = {}


def _build_bass():
    import concourse.bass as bass
    import concourse.tile as tile
    from concourse import mybir

    f8 = mybir.dt.float8e4
    f16 = mybir.dt.float16
    f32 = mybir.dt.float32
    DR = mybir.MatmulPerfMode.DoubleRow
    nc = bass.Bass()

    # Per-core input: host-preswizzled fp8 DMA regions, concatenated flat in
    # DMA issue order. Each region is [128, nu, 784] partition-major (rows of
    # nu*784 bytes), where a "unit" is one (layer, chunk) slice: 768 hs cols,
    # 2 mask cols, 14 pad (16-aligned for DoubleRow's step%16==0 rule).
    n_units = B_SHARD * 2 * HALF_UNITS  # 64
    hsd = nc.dram_tensor(
        "hsd", [n_units * 128 * CH_COLS], f8, kind="ExternalInput"
    )
    # m-major output; host transposes to [B_SHARD, 2H].
    out = nc.dram_tensor("out", [2, B_SHARD * H], f16, kind="ExternalOutput")

    with tile.TileContext(nc) as tc:
        with (
            tc.tile_pool(name="hs_pool", bufs=1) as hs_pool,
            tc.tile_pool(name="out_pool", bufs=1) as out_pool,
            tc.tile_pool(name="psum", bufs=4, space="PSUM") as psum_pool,
        ):
            out_tile = out_pool.tile([2, B_SHARD * H], f16)

            # All hs DMAs ride the SYNC HWDGE ring: one queue alone
            # saturates all 16 SDMA engines (~430GB/s measured), it starts
            # ~2us before the scalar ring, and single-queue FIFO makes
            # arrivals match PE consumption order exactly. 7 hs DMAs +
            # the out store = the 8 HWDGE sem lanes (a 9th would wrap a
            # lane and need a 2nd sync wait). b0/b1 ship as full blobs (PE
            # has slack at the start); b3's tail ships as two 4-unit
            # quarters so only ~0.7us of matmuls depend on the last DMA.
            off = [0]

            def hs_dma(nu, tag):
                t = hs_pool.tile([128, nu, CH_COLS], f8, tag=tag)
                sz = 128 * nu * CH_COLS
                nc.sync.dma_start(
                    out=t[:],
                    in_=hsd[off[0] : off[0] + sz].rearrange(
                        "(p u c) -> p u c", p=128, c=CH_COLS
                    ),
                )
                off[0] += sz
                return t

            split = {0: (16,), 1: (16,), 2: (8, 8), 3: (8, 4, 4)}

            for b in range(B_SHARD):
                mm_args = []
                for di, nu in enumerate(split[b]):
                    t = hs_dma(nu, f"hs{b}_{di}")
                    for up in range(0, nu, 2):  # chunk pairs (DoubleRow K=256)
                        lhsT = t[:, up : up + 2, H : H + 2]  # step %16==0
                        mm_args.append(
                            (lhsT, t[:, up : up + 2, 0:512],
                             t[:, up : up + 2, 512:H])
                        )

                # One [2, 1024] psum tile = exactly 2 banks: bank A holds
                # cols 0:512, bank B cols 512:768 (each matmul's out stays
                # inside one bank). 4 batches use all 8 banks.
                psum_t = psum_pool.tile([2, 1024], f32, tag="psum")
                n = len(mm_args)
                for i, (lhsT, rhs_a, rhs_b) in enumerate(mm_args):
                    nc.tensor.matmul(psum_t[:, 0:512], lhsT, rhs_a,
                                     start=i == 0, stop=i == n - 1,
                                     perf_mode=DR)
                    nc.tensor.matmul(psum_t[:, 512:H], lhsT, rhs_b,
                                     start=i == 0, stop=i == n - 1,
                                     perf_mode=DR)
                # Single f32->f16 cast per batch (one DVE instruction).
                nc.vector.tensor_copy(
                    out=out_tile[:, b * H : (b + 1) * H], in_=psum_t[:, 0:H]
                )

            # Output store is the scalar HWDGE ring's ONLY DMA: no sem-lane
            # wrap, so its single sync wait is the DVE-copies semaphore.
            nc.scalar.dma_start(out=out[:], in_=out_tile[:])

    _fix_drain_waits(nc)
    return nc


def _fix_drain_waits(nc):
    """This container's walrus accepts only ONE sync wait per instruction;
    Tile's exit drain aggregates one wait per live semaphore. In this kernel
    every semaphore except the final out-DMA's is transitively ordered before
    the drain (matmuls wait on hs DMAs -> PE; copies wait on PE -> DVE; the
    out DMA waits on DVE; the drain runs after on the same SP queue), so the
    drain only truly needs the out-DMA completion wait.
    """
    import bass_rust

    f = nc.m.functions[0]
    # update-sem of the last DMACopy in program order (the out store)
    last_dma_sem = None
    for bb in f.blocks:
        for ins in bb.instructions:
            if type(ins).__name__ == "InstDMACopy":
                ups = ins.sync_info.on_update
                if ups:
                    last_dma_sem = ups[-1].ant_name

    for bb in f.blocks:
        for ins in bb.instructions:
            if type(ins).__name__ != "InstDrain":
                continue
            si = ins.sync_info
            if si is None:
                continue
            waits = list(si.on_wait)
            if len(waits) <= 1:
                continue
            keep = [w for w in waits if w.ant_name == last_dma_sem]
            assert len(keep) == 1, (last_dma_sem, [w.ant_name for w in waits])
            ins.sync_info = bass_rust.SyncInfo(
                on_wait=keep, on_update=list(si.on_update)
            )


def _host_masks(input_ids, attention_mask, token_type_ids):
    ids = np.asarray(input_ids)
    am = np.asarray(attention_mask)
    tt = np.asarray(token_type_ids)

    not_pad = ids != PAD_ID
    before_pad = np.cumprod(not_pad.astype(np.int64), axis=1).astype(bool)
    valid = before_pad & (ids != CLS_ID) & (ids != SEP_ID) & (am == 1)
    term = valid & (tt == 0)
    text = valid & (tt == 1)
    masks = np.stack([term, text], axis=-1)  # [B, S, 2] bool
    counts = masks.sum(axis=1).astype(np.float64)  # [B, 2]
    return masks, counts


def _compensated_fp8(hs4, masks, f8):
    """Quantize to fp8 e4m3 with per-group error diffusion along the
    reduction axis: the rounding residual of each masked element is carried
    into the next masked element of the SAME group's (b, h) chain, so each
    group's quantization errors telescope to ~1 ulp instead of a sqrt(N)
    random walk. Device-side sum order doesn't matter -- only the group SUM
    of the quantized values.
    """
    q = hs4.astype(f8)  # [4, B, S, H]; unmasked positions: plain rounding
    g0_all = masks[:, :, 0]
    g1_all = masks[:, :, 1]
    carry = np.zeros((2, B, H), dtype=np.float32)
    for l in range(N_LAYERS):
        for s in range(S):
            g0 = g0_all[:, s]
            g1 = g1_all[:, s]
            if not (g0.any() or g1.any()):
                continue
            gany = (g0 | g1)[:, None]
            c = np.where(g0[:, None], carry[0], carry[1])
            t = hs4[l, :, s, :] + c
            qv = t.astype(f8)
            q[l, :, s, :] = np.where(gany, qv, q[l, :, s, :])
            resid = t - qv.astype(np.float32)
            carry[0] = np.where(g0[:, None], resid, carry[0])
            carry[1] = np.where(g1[:, None], resid, carry[1])
    return q


def kernel(hidden_states, input_ids, attention_mask, token_type_ids):
    import ml_dtypes
    from concourse.bass_utils import run_bass_kernel_spmd

    f8 = ml_dtypes.float8_e4m3fn  # same encoding as TRN e4m3 for |x| <= 240

    hs_full = np.asarray(hidden_states)
    masks, counts = _host_masks(input_ids, attention_mask, token_type_ids)

    q = _compensated_fp8(
        hs_full[L - N_LAYERS :].astype(np.float32), masks, f8
    )  # [4, B, S, H] fp8

    # Per-batch unit array U[b, p, l*4+c, col]: hs cols 0:768, mask cols
    # 768:770, pad to 784. Then concatenate [128, nu, 784] p-major DMA
    # regions in issue order (must mirror _build_bass's split dict).
    U = np.empty((B, 128, 2 * HALF_UNITS, CH_COLS), dtype=f8)
    U[..., :H] = (
        q.reshape(N_LAYERS, B, N_CHUNKS, 128, H)
        .transpose(1, 3, 0, 2, 4)
        .reshape(B, 128, 2 * HALF_UNITS, H)
    )
    m8 = masks.astype(f8).reshape(B, N_CHUNKS, 128, 2).transpose(0, 2, 1, 3)
    U[..., H : H + 2] = np.broadcast_to(
        m8[:, :, None, :, :], (B, 128, N_LAYERS, N_CHUNKS, 2)
    ).reshape(B, 128, 2 * HALF_UNITS, 2)
    U[..., H + 2 :] = np.zeros((), dtype=f8)  # pad cols, never read

    split = {0: (16,), 1: (16,), 2: (8, 8), 3: (8, 4, 4)}
    in_maps = []
    for i in range(N_CORES):
        parts = []
        for b in range(B_SHARD):
            u0 = 0
            for nu in split[b]:
                parts.append(
                    U[i * B_SHARD + b, :, u0 : u0 + nu, :].reshape(-1)
                )
                u0 += nu
        in_maps.append({"hsd": np.concatenate(parts)})

    if "nc" not in _CACHED:
        _CACHED["nc"] = _build_bass()
    nc = _CACHED["nc"]

    trace = os.environ.get("KERNEL_TRACE", "0") == "1"
    if trace:
        _install_ntff_hook_shim()
    tmpdir = os.environ.get("KERNEL_TMPDIR") or None
    res = run_bass_kernel_spmd(
        nc, in_maps, core_ids=list(range(N_CORES)), trace=trace, tmpdir=tmpdir
    )
    kernel.last_results = res

    # [2, B_SHARD*H] fp16 per core -> [B, 2, H] f32
    acc = np.concatenate(
        [r["out"].reshape(2, B_SHARD, H).transpose(1, 0, 2) for r in res.results],
        axis=0,
    ).astype(np.float32)
    # Apply the masked-mean normalization (exact f64 scale, mirrors the
    # reference's sum/count including inf/nan semantics for count==0).
    with np.errstate(divide="ignore", invalid="ignore"):
        scale = 1.0 / (N_LAYERS * counts)  # [B, 2]
    out = acc * scale[:, :, None]
    return out.reshape(B, 2 * H).astype(np.float32)


def _install_ntff_hook_shim():
    """The container's antenv stub lacks axon_hooks, which silently disables
    NTFF profiling under trace=True. Recreate it: a tiny get/set registry plus
    the ctypes hook into libaxon_pjrt.so (same as trn_boot's installer)."""
    import contextlib
    import ctypes
    import sys
    import types

    if "antenv.axon_hooks" in sys.modules:
        return
    so_path = "/opt/axon/libaxon_pjrt.so"
    try:
        lib = ctypes.CDLL(so_path)
    except OSError:
        return
    if not hasattr(lib, "axon_start_nrt_profile"):
        return
    lib.axon_start_nrt_profile.argtypes = [
        ctypes.POINTER(ctypes.c_int64),
        ctypes.c_size_t,
    ]
    lib.axon_start_nrt_profile.restype = ctypes.c_int64
    lib.axon_stop_nrt_profile.argtypes = [ctypes.c_char_p]
    lib.axon_stop_nrt_profile.restype = ctypes.c_int64

    @contextlib.contextmanager
    def _hook(output_dir, device_ids):
        import jax

        jax.devices()
        if device_ids:
            ids = (ctypes.c_int64 * len(device_ids))(*device_ids)
            rc = lib.axon_start_nrt_profile(ids, len(device_ids))
        else:
            rc = lib.axon_start_nrt_profile(None, 0)
        if rc != 0:
            raise RuntimeError(f"axon_start_nrt_profile rc={rc}")
        try:
            yield
        finally:
            n = lib.axon_stop_nrt_profile(str(output_dir).encode())
            print(f"profile: {n} file(s) written to {output_dir}", file=sys.stderr)

    mod = types.ModuleType("antenv.axon_hooks")
    _state = {"hook": _hook}
    mod.set_axon_ntff_profile_hook = lambda h: _state.__setitem__("hook", h)
    mod.get_axon_ntff_profile_hook = lambda: _state["hook"]
    sys.modules["antenv.axon_hooks"] = mod
    import antenv

    antenv.axon_hooks = mod


# revision 16
# speedup vs baseline: 1.6674x; 1.0103x over previous
"""Bass/Trainium2 kernel for nn_CustomPooling (segment_reduce, masked mean pooling).

Reference computation:
  hs = mean(hidden_states[-4:], axis=0)                      # [B,S,H]
  valid = before_pad & ~CLS & ~SEP & attention
  term_mean = sum_s(hs * term_mask) / sum(term_mask)         # [B,H]
  text_mean = sum_s(hs * text_mask) / sum(text_mask)         # [B,H]
  out = concat([term_mean, text_mean], -1)                   # [B,2H]

Strategy (v2, fp8):
  - Only the last 4 layers are ever read (201MB of the 654MB input).
  - The [B,S] int masks reduce to binary {0,1} per-(b,s) weights; the
    1/(4*count) scale is applied to the tiny [B,2H] result on the host, so
    the device work is a pure masked sum over (layer, s):
      acc[b, m*H + h] = sum_{l,s} hs[l,b,s,h] * mask[b,s,m]
  - Data ships as fp8 e4m3 ({0,1} masks exact; hs quantized with per-group
    error diffusion so each group's rounding errors telescope to ~1 ulp).
    This halves DMA bytes vs the fp16 version: 6.3MB/core, ~15us at the
    ~427GB/s the 16 SDMA engines sustain.
  - The reduction is a TensorE matmul in fp8 DoubleRow perf mode: one
    instruction contracts TWO 128-row s-chunks (K=256) at the double-pumped
    fp8 rate. lhsT = [128,2,2] mask slice, rhs = [128,2,N] hs slice.
  - Data parallel over B: 8 cores x 4 batches, no collectives.
  - Host pre-swizzles each (batch, layer-pair) into one contiguous
    [128, 8, 770] fp8 half-blob (per (layer,chunk) unit: 768 hs cols + its
    own 2 mask cols) so every matmul waits on exactly one DMA semaphore and
    per-partition DMA rows are 6160B (full 26GB/s per-engine packet rate;
    3KB rows drop to ~18GB/s). 8 half-blob DMAs alternate between the two
    HWDGE rings (sync/scalar) to keep all 16 SDMA engines saturated.
  - PSUM->SBUF copies cast to fp16 (2x DVE rate, ~1e-4 rel err); the single
    output store is one contiguous HWDGE DMA of the m-major [2,B/8*H] tile;
    the host transposes to [B/8, 2H] and applies the f64 1/(4*count) scale.
"""

import os

import numpy as np

# Hardcoded problem shape (kernel.py must be self-contained).
L, B, S, H = 13, 32, 512, 768
N_LAYERS = 4          # layers -4..-1
N_CORES = 8
B_SHARD = B // N_CORES          # 4 batches per core
N_CHUNKS = S // 128             # 4 s-chunks of 128 (PE contraction dim)
CH_COLS = H + 16                # 784 (16-aligned: DoubleRow needs k-tile step%16==0);
                                # per unit: 768 hs cols, 2 mask cols, 14 pad
HALF_UNITS = 2 * N_CHUNKS       # 8 units per half-blob (2 layers x 4 chunks)
CLS_ID, SEP_ID, PAD_ID = 101, 102, 0

_CACHED = {}


def _build_bass():
    import concourse.bass as bass
    import concourse.tile as tile
    from concourse import mybir

    f8 = mybir.dt.float8e4
    f16 = mybir.dt.float16
    f32 = mybir.dt.float32
    DR = mybir.MatmulPerfMode.DoubleRow
    nc = bass.Bass()

    # Per-core input: host-preswizzled fp8 DMA regions, concatenated flat in
    # DMA issue order. Each region is [128, nu, 784] partition-major (rows of
    # nu*784 bytes), where a "unit" is one (layer, chunk) slice: 768 hs cols,
    # 2 mask cols, 14 pad (16-aligned for DoubleRow's step%16==0 rule).
    n_units = B_SHARD * 2 * HALF_UNITS  # 64
    # +8KB zero tail: warmup scratch, fetched by the scalar-ring warm DMA.
    hsd = nc.dram_tensor(
        "hsd", [n_units * 128 * CH_COLS + 128 * 64], f8, kind="ExternalInput"
    )
    # m-major output; host transposes to [B_SHARD, 2H].
    out = nc.dram_tensor("out", [2, B_SHARD * H], f16, kind="ExternalOutput")

    with tile.TileContext(nc) as tc:
        with (
            tc.tile_pool(name="hs_pool", bufs=1) as hs_pool,
            tc.tile_pool(name="out_pool", bufs=1) as out_pool,
            tc.tile_pool(name="psum", bufs=4, space="PSUM") as psum_pool,
        ):
            out_tile = out_pool.tile([2, B_SHARD * H], f16)

            # All hs DMAs ride the SYNC HWDGE ring: one queue alone
            # saturates all 16 SDMA engines (~430GB/s measured), it starts
            # ~2us before the scalar ring, and single-queue FIFO makes
            # arrivals match PE consumption order exactly. 7 hs DMAs +
            # the out store = the 8 HWDGE sem lanes (a 9th would wrap a
            # lane and need a 2nd sync wait). b0/b1 ship as full blobs (PE
            # has slack at the start); b3's tail ships as two 4-unit
            # quarters so only ~0.7us of matmuls depend on the last DMA.
            off = [0]

            def hs_dma(nu, tag):
                t = hs_pool.tile([128, nu, CH_COLS], f8, tag=tag)
                sz = 128 * nu * CH_COLS
                nc.sync.dma_start(
                    out=t[:],
                    in_=hsd[off[0] : off[0] + sz].rearrange(
                        "(p u c) -> p u c", p=128, c=CH_COLS
                    ),
                )
                off[0] += sz
                return t

            split = {0: (4, 12), 1: (16,), 2: (16,), 3: (8, 8)}

            # PE clock warmup: the HAM clock-gates a cold TensorE to half
            # rate and only ramps after sustained activity (~40% of the v3
            # run executed throttled). Run dummy DoubleRow matmuls on a
            # zero scratch tile while the first hs DMA is in flight; they
            # write b0's psum tile, whose first REAL matmul (start=True)
            # resets the accumulation, so the garbage never escapes. The
            # scratch load is the scalar HWDGE ring's first DMA, which also
            # warms that ring for the final store (a cold ring's first
            # descriptor costs ~2us).
            scratch = hs_pool.tile([128, 2, 32], f8, tag="warm")
            zoff = n_units * 128 * CH_COLS
            nc.scalar.dma_start(
                out=scratch[:],
                in_=hsd[zoff : zoff + 128 * 64].rearrange(
                    "(p u c) -> p u c", p=128, c=32
                ),
            )
            psum_first = psum_pool.tile([2, 1024], f32, tag="psum")
            N_WARM = 24
            for i in range(N_WARM):
                nc.tensor.matmul(psum_first[:, 0:32], scratch[:, :, 0:2],
                                 scratch[:], start=i == 0, stop=i == N_WARM - 1,
                                 perf_mode=DR)

            for b in range(B_SHARD):
                mm_args = []
                for di, nu in enumerate(split[b]):
                    t = hs_dma(nu, f"hs{b}_{di}")
                    for up in range(0, nu, 2):  # chunk pairs (DoubleRow K=256)
                        lhsT = t[:, up : up + 2, H : H + 2]  # step %16==0
                        mm_args.append(
                            (lhsT, t[:, up : up + 2, 0:512],
                             t[:, up : up + 2, 512:H])
                        )

                # One [2, 1024] psum tile = exactly 2 banks: bank A holds
                # cols 0:512, bank B cols 512:768 (each matmul's out stays
                # inside one bank). 4 batches use all 8 banks.
                psum_t = psum_first if b == 0 else psum_pool.tile(
                    [2, 1024], f32, tag="psum"
                )
                n = len(mm_args)
                for i, (lhsT, rhs_a, rhs_b) in enumerate(mm_args):
                    nc.tensor.matmul(psum_t[:, 0:512], lhsT, rhs_a,
                                     start=i == 0, stop=i == n - 1,
                                     perf_mode=DR)
                    nc.tensor.matmul(psum_t[:, 512:H], lhsT, rhs_b,
                                     start=i == 0, stop=i == n - 1,
                                     perf_mode=DR)
                # Single f32->f16 cast per batch (one DVE instruction).
                nc.vector.tensor_copy(
                    out=out_tile[:, b * H : (b + 1) * H], in_=psum_t[:, 0:H]
                )
            # Single full output store on the (pre-warmed) scalar ring.
            # warm + 6 hs + out = the 8 HWDGE sem lanes (a 9th DMA would
            # wrap a lane and need a 2nd sync wait).
            nc.scalar.dma_start(out=out[:], in_=out_tile[:])

    _fix_drain_waits(nc)
    return nc


def _fix_drain_waits(nc):
    """This container's walrus accepts only ONE sync wait per instruction;
    Tile's exit drain aggregates one wait per live semaphore. In this kernel
    every semaphore except the final out-DMA's is transitively ordered before
    the drain (matmuls wait on hs DMAs -> PE; copies wait on PE -> DVE; the
    out DMA waits on DVE; the drain runs after on the same SP queue), so the
    drain only truly needs the out-DMA completion wait.
    """
    import bass_rust

    f = nc.m.functions[0]
    # update-sem of the last DMACopy in program order (the out store)
    last_dma_sem = None
    for bb in f.blocks:
        for ins in bb.instructions:
            if type(ins).__name__ == "InstDMACopy":
                ups = ins.sync_info.on_update
                if ups:
                    last_dma_sem = ups[-1].ant_name

    for bb in f.blocks:
        for ins in bb.instructions:
            if type(ins).__name__ != "InstDrain":
                continue
            si = ins.sync_info
            if si is None:
                continue
            waits = list(si.on_wait)
            if len(waits) <= 1:
                continue
            keep = [w for w in waits if w.ant_name == last_dma_sem]
            assert len(keep) == 1, (last_dma_sem, [w.ant_name for w in waits])
            ins.sync_info = bass_rust.SyncInfo(
                on_wait=keep, on_update=list(si.on_update)
            )


def _host_masks(input_ids, attention_mask, token_type_ids):
    ids = np.asarray(input_ids)
    am = np.asarray(attention_mask)
    tt = np.asarray(token_type_ids)

    not_pad = ids != PAD_ID
    before_pad = np.cumprod(not_pad.astype(np.int64), axis=1).astype(bool)
    valid = before_pad & (ids != CLS_ID) & (ids != SEP_ID) & (am == 1)
    term = valid & (tt == 0)
    text = valid & (tt == 1)
    masks = np.stack([term, text], axis=-1)  # [B, S, 2] bool
    counts = masks.sum(axis=1).astype(np.float64)  # [B, 2]
    return masks, counts


def _compensated_fp8(hs4, masks, f8):
    """Quantize to fp8 e4m3 with per-group error diffusion along the
    reduction axis: the rounding residual of each masked element is carried
    into the next masked element of the SAME group's (b, h) chain, so each
    group's quantization errors telescope to ~1 ulp instead of a sqrt(N)
    random walk. Device-side sum order doesn't matter -- only the group SUM
    of the quantized values.
    """
    q = hs4.astype(f8)  # [4, B, S, H]; unmasked positions: plain rounding
    g0_all = masks[:, :, 0]
    g1_all = masks[:, :, 1]
    carry = np.zeros((2, B, H), dtype=np.float32)
    for l in range(N_LAYERS):
        for s in range(S):
            g0 = g0_all[:, s]
            g1 = g1_all[:, s]
            if not (g0.any() or g1.any()):
                continue
            gany = (g0 | g1)[:, None]
            c = np.where(g0[:, None], carry[0], carry[1])
            t = hs4[l, :, s, :] + c
            qv = t.astype(f8)
            q[l, :, s, :] = np.where(gany, qv, q[l, :, s, :])
            resid = t - qv.astype(np.float32)
            carry[0] = np.where(g0[:, None], resid, carry[0])
            carry[1] = np.where(g1[:, None], resid, carry[1])
    return q


def kernel(hidden_states, input_ids, attention_mask, token_type_ids):
    import ml_dtypes
    from concourse.bass_utils import run_bass_kernel_spmd

    f8 = ml_dtypes.float8_e4m3fn  # same encoding as TRN e4m3 for |x| <= 240

    hs_full = np.asarray(hidden_states)
    masks, counts = _host_masks(input_ids, attention_mask, token_type_ids)

    q = _compensated_fp8(
        hs_full[L - N_LAYERS :].astype(np.float32), masks, f8
    )  # [4, B, S, H] fp8

    # Per-batch unit array U[b, p, l*4+c, col]: hs cols 0:768, mask cols
    # 768:770, pad to 784. Then concatenate [128, nu, 784] p-major DMA
    # regions in issue order (must mirror _build_bass's split dict).
    U = np.empty((B, 128, 2 * HALF_UNITS, CH_COLS), dtype=f8)
    U[..., :H] = (
        q.reshape(N_LAYERS, B, N_CHUNKS, 128, H)
        .transpose(1, 3, 0, 2, 4)
        .reshape(B, 128, 2 * HALF_UNITS, H)
    )
    m8 = masks.astype(f8).reshape(B, N_CHUNKS, 128, 2).transpose(0, 2, 1, 3)
    U[..., H : H + 2] = np.broadcast_to(
        m8[:, :, None, :, :], (B, 128, N_LAYERS, N_CHUNKS, 2)
    ).reshape(B, 128, 2 * HALF_UNITS, 2)
    U[..., H + 2 :] = np.zeros((), dtype=f8)  # pad cols, never read

    split = {0: (4, 12), 1: (16,), 2: (16,), 3: (8, 8)}
    ztail = np.zeros(128 * 64, dtype=f8)  # warmup scratch (see _build_bass)
    in_maps = []
    for i in range(N_CORES):
        parts = []
        for b in range(B_SHARD):
            u0 = 0
            for nu in split[b]:
                parts.append(
                    U[i * B_SHARD + b, :, u0 : u0 + nu, :].reshape(-1)
                )
                u0 += nu
        parts.append(ztail)
        in_maps.append({"hsd": np.concatenate(parts)})

    if "nc" not in _CACHED:
        _CACHED["nc"] = _build_bass()
    nc = _CACHED["nc"]

    trace = os.environ.get("KERNEL_TRACE", "0") == "1"
    if trace:
        _install_ntff_hook_shim()
    tmpdir = os.environ.get("KERNEL_TMPDIR") or None
    res = run_bass_kernel_spmd(
        nc, in_maps, core_ids=list(range(N_CORES)), trace=trace, tmpdir=tmpdir
    )
    kernel.last_results = res

    # [2, B_SHARD*H] fp16 per core -> [B, 2, H] f32
    acc = np.concatenate(
        [r["out"].reshape(2, B_SHARD, H).transpose(1, 0, 2) for r in res.results],
        axis=0,
    ).astype(np.float32)
    # Apply the masked-mean normalization (exact f64 scale, mirrors the
    # reference's sum/count including inf/nan semantics for count==0).
    with np.errstate(divide="ignore", invalid="ignore"):
        scale = 1.0 / (N_LAYERS * counts)  # [B, 2]
    out = acc * scale[:, :, None]
    return out.reshape(B, 2 * H).astype(np.float32)


def _install_ntff_hook_shim():
    """The container's antenv stub lacks axon_hooks, which silently disables
    NTFF profiling under trace=True. Recreate it: a tiny get/set registry plus
    the ctypes hook into libaxon_pjrt.so (same as trn_boot's installer)."""
    import contextlib
    import ctypes
    import sys
    import types

    if "antenv.axon_hooks" in sys.modules:
        return
    so_path = "/opt/axon/libaxon_pjrt.so"
    try:
        lib = ctypes.CDLL(so_path)
    except OSError:
        return
    if not hasattr(lib, "axon_start_nrt_profile"):
        return
    lib.axon_start_nrt_profile.argtypes = [
        ctypes.POINTER(ctypes.c_int64),
        ctypes.c_size_t,
    ]
    lib.axon_start_nrt_profile.restype = ctypes.c_int64
    lib.axon_stop_nrt_profile.argtypes = [ctypes.c_char_p]
    lib.axon_stop_nrt_profile.restype = ctypes.c_int64

    @contextlib.contextmanager
    def _hook(output_dir, device_ids):
        import jax

        jax.devices()
        if device_ids:
            ids = (ctypes.c_int64 * len(device_ids))(*device_ids)
            rc = lib.axon_start_nrt_profile(ids, len(device_ids))
        else:
            rc = lib.axon_start_nrt_profile(None, 0)
        if rc != 0:
            raise RuntimeError(f"axon_start_nrt_profile rc={rc}")
        try:
            yield
        finally:
            n = lib.axon_stop_nrt_profile(str(output_dir).encode())
            print(f"profile: {n} file(s) written to {output_dir}", file=sys.stderr)

    mod = types.ModuleType("antenv.axon_hooks")
    _state = {"hook": _hook}
    mod.set_axon_ntff_profile_hook = lambda h: _state.__setitem__("hook", h)
    mod.get_axon_ntff_profile_hook = lambda: _state["hook"]
    sys.modules["antenv.axon_hooks"] = mod
    import antenv

    antenv.axon_hooks = mod
